# revision 2
# baseline (speedup 1.0000x reference)
import numpy as np
import jax
import jax.numpy as jnp

# Problem shapes (hardcoded per spec)
B = 64       # graphs
NPG = 800    # nodes per graph
EPG = 6400   # edges per graph
APG = 64     # actions per graph
H = 4        # heads
C = 64       # channels per head
OUT = 128    # out channels
M = 8        # cores
GPC = B // M         # graphs per core = 8
NL = GPC * NPG       # nodes per core = 6400
EL = GPC * EPG       # edges per core = 51200
AL = GPC * APG       # actions per core = 512


def _gat_dense(h, A_cnt, mask, W, a_src, a_dst, bias, concat):
    """Dense per-graph GAT.

    h:     [NL, fin] node features (graphs are contiguous blocks of NPG)
    A_cnt: [GPC, NPG, NPG] edge multiplicity counts, A_cnt[g, d, s] (incl. self loop)
    mask:  [GPC, NPG, NPG] bool, A_cnt > 0
    """
    Hh, Cc = a_src.shape
    xp = (h @ W).reshape(NL, Hh, Cc)                       # [NL,H,C]
    al_s = (xp * a_src).sum(-1).reshape(GPC, NPG, Hh)      # [G,N,H]
    al_d = (xp * a_dst).sum(-1).reshape(GPC, NPG, Hh)
    # logits z[g,h,d,s] = leaky_relu(al_s[g,s,h] + al_d[g,d,h])
    z = al_s.transpose(0, 2, 1)[:, :, None, :] + al_d.transpose(0, 2, 1)[:, :, :, None]
    z = jax.nn.leaky_relu(z, 0.2)                          # [G,H,D,S]
    neg = jnp.float32(-1e30)
    zm = jnp.where(mask[:, None], z, neg)
    m = zm.max(axis=-1, keepdims=True)                     # [G,H,D,1]
    ex = jnp.exp(z - m) * A_cnt[:, None]                   # weighted by multiplicity
    den = ex.sum(axis=-1)                                  # [G,H,D]
    xph = xp.reshape(GPC, NPG, Hh, Cc)
    out = jnp.einsum('ghds,gshc->gdhc', ex, xph)
    out = out / (den + 1e-16).transpose(0, 2, 1)[..., None]
    out = out.reshape(NL, Hh, Cc)
    out = out.reshape(NL, Hh * Cc) if concat else out.mean(axis=1)
    return out + bias


def _forward(x, A_fwd, mask_fwd, A_bwd, mask_bwd, actions_idx, tabu_label,
             f1_W, f1_as, f1_ad, f1_b, f2_W, f2_as, f2_ad, f2_b,
             f3_W, f3_as, f3_ad, f3_b, b1_W, b1_as, b1_ad, b1_b,
             b2_W, b2_as, b2_ad, b2_b, b3_W, b3_as, b3_ad, b3_b,
             p_W1, p_b1, p_W2, p_b2, p_W3, p_b3, p_W4, p_b4):
    hf = x[:, jnp.array([0, 1, 3])]
    hf = jax.nn.elu(_gat_dense(hf, A_fwd, mask_fwd, f1_W, f1_as, f1_ad, f1_b, True))
    hf = jax.nn.elu(_gat_dense(hf, A_fwd, mask_fwd, f2_W, f2_as, f2_ad, f2_b, True))
    hf3 = _gat_dense(hf, A_fwd, mask_fwd, f3_W, f3_as, f3_ad, f3_b, False)
    hb = x[:, jnp.array([0, 2, 4])]
    hb = jax.nn.elu(_gat_dense(hb, A_bwd, mask_bwd, b1_W, b1_as, b1_ad, b1_b, True))
    hb = jax.nn.elu(_gat_dense(hb, A_bwd, mask_bwd, b2_W, b2_as, b2_ad, b2_b, True))
    hb3 = _gat_dense(hb, A_bwd, mask_bwd, b3_W, b3_as, b3_ad, b3_b, False)
    h_node = jnp.concatenate([hf3, hb3], axis=-1)          # [NL, 2*OUT]
    g_pool = h_node.reshape(GPC, NPG, 2 * OUT).mean(axis=1)  # [G, 2*OUT]
    g_exp = jnp.repeat(g_pool, NPG, axis=0)                # [NL, 2*OUT] (contiguous graphs)
    node_h = jnp.concatenate([h_node, g_exp], axis=-1)     # [NL, 4*OUT]
    action_h = jnp.concatenate([node_h[actions_idx[:, 0]],
                                node_h[actions_idx[:, 1]],
                                tabu_label], axis=-1)
    h = jnp.tanh(action_h @ p_W1 + p_b1)
    h = jnp.tanh(h @ p_W2 + p_b2)
    h = jnp.tanh(h @ p_W3 + p_b3)
    score = (h @ p_W4 + p_b4).reshape(GPC, APG)
    log_prob = jax.nn.log_softmax(score, axis=-1)
    entropy = -(jnp.exp(log_prob) * log_prob).sum(-1, keepdims=True)
    return log_prob, entropy


def _build_adj(src, dst):
    """Per-device [GPC, NPG, NPG] multiplicity matrix from local edge lists.

    A[g, d, s] = #edges s->d in graph g, +1 on the diagonal (self loops).
    Built with a flat scalar scatter-add (device-side)."""
    g = dst // NPG
    flat = g * (NPG * NPG) + (dst - g * NPG) * NPG + (src - g * NPG)
    A = jnp.zeros((GPC * NPG * NPG,), jnp.float32).at[flat].add(1.0)
    A = A.reshape(GPC, NPG, NPG) + jnp.eye(NPG, dtype=jnp.float32)[None]
    return A


_pA = None
_pF = None


def _get_fns():
    global _pA, _pF
    if _pA is None:
        devs = jax.devices()[:M]
        _pA = jax.pmap(_build_adj, devices=devs)
        _pF = jax.pmap(
            _forward,
            in_axes=(0, 0, 0, 0, 0, 0, 0) + (None,) * 32,
            devices=devs,
        )
    return _pA, _pF


_WNAMES = ['f1_W', 'f1_as', 'f1_ad', 'f1_b', 'f2_W', 'f2_as', 'f2_ad', 'f2_b',
           'f3_W', 'f3_as', 'f3_ad', 'f3_b', 'b1_W', 'b1_as', 'b1_ad', 'b1_b',
           'b2_W', 'b2_as', 'b2_ad', 'b2_b', 'b3_W', 'b3_as', 'b3_ad', 'b3_b',
           'p_W1', 'p_b1', 'p_W2', 'p_b2', 'p_W3', 'p_b3', 'p_W4', 'p_b4']


def kernel(**inputs):
    x = np.asarray(inputs['x']).reshape(M, NL, 5)
    ei = np.asarray(inputs['edge_index'])
    base = (np.arange(M, dtype=np.int32) * NL)[:, None]
    src = (ei[0].reshape(M, EL) - base).astype(np.int32)
    dst = (ei[1].reshape(M, EL) - base).astype(np.int32)
    act = (np.asarray(inputs['actions_idx']).reshape(M, AL, 2)
           - base[:, :, None]).astype(np.int32)
    tabu = np.asarray(inputs['tabu_label']).reshape(M, AL, 1)
    weights = [inputs[k] for k in _WNAMES]

    pA, pF = _get_fns()
    A_fwd = pA(src, dst)                 # aggregate over dst (forward flow)
    A_bwd = pA(dst, src)                 # reversed edges
    m_fwd = A_fwd > 0
    m_bwd = A_bwd > 0
    log_prob, entropy = pF(x, A_fwd, m_fwd, A_bwd, m_bwd, act, tabu, *weights)
    log_prob = np.asarray(log_prob).reshape(B, APG)
    entropy = np.asarray(entropy).reshape(B, 1)
    return log_prob, entropy


# revision 6
# speedup vs baseline: 1.0550x; 1.0550x over previous
import numpy as np
import jax
import jax.numpy as jnp

# Problem shapes (hardcoded per spec)
B = 64       # graphs
NPG = 800    # nodes per graph
EPG = 6400   # edges per graph
APG = 64     # actions per graph
H = 4        # heads
C = 64       # channels per head
OUT = 128    # out channels
M = 8        # cores
GPC = B // M         # graphs per core = 8
NL = GPC * NPG       # nodes per core = 6400
EL = GPC * EPG       # edges per core = 51200
AL = GPC * APG       # actions per core = 512


def _gat_dense(h, A, W, a_src, a_dst, bias, concat):
    """Dense per-graph GAT, exact (multiplicity-weighted, shift-invariant softmax).

    h: [NL, fin] node features (graphs contiguous blocks of NPG)
    A: [GPC, NPG, NPG] edge multiplicity counts A[g, d, s] (incl. self loops)
    """
    Hh, Cc = a_src.shape
    GB = GPC * Hh
    xp = (h @ W).reshape(NL, Hh, Cc)                       # [NL,H,C]
    al_s = (xp * a_src).sum(-1).reshape(GPC, NPG, Hh)      # [G,N,H]
    al_d = (xp * a_dst).sum(-1).reshape(GPC, NPG, Hh)
    alsT = al_s.transpose(0, 2, 1).reshape(GB, 1, NPG)     # [GB,1,S]
    aldT = al_d.transpose(0, 2, 1).reshape(GB, NPG, 1)     # [GB,D,1]
    # per-dst upper bound on the row logits: mhat >= lr(al_s[s]+al_d[d]) for all s
    gms = al_s.max(axis=1).transpose(0, 1).reshape(GPC, 1, Hh)   # max over nodes [G,1,H]
    mhat = jax.nn.leaky_relu(gms.transpose(0, 2, 1).reshape(GB, 1, 1)
                             + aldT, 0.2)                  # [GB,D,1]
    z = jax.nn.leaky_relu(aldT + alsT, 0.2)                # [GB,D,S]
    ex = jnp.exp(z - mhat)                                 # <= 1
    ex = ex * jnp.repeat(A, Hh, axis=0)                    # multiplicity weight / mask
    # aggregate: out[b,d,c] = sum_s ex[b,d,s] xp[b,s,c]; ones column -> denominator
    xph = xp.reshape(GPC, NPG, Hh, Cc).transpose(0, 2, 1, 3).reshape(GB, NPG, Cc)
    xph1 = jnp.concatenate([xph, jnp.ones((GB, NPG, 1), xph.dtype)], axis=-1)
    agg = jnp.matmul(ex, xph1)                             # [GB,D,C+1]
    out = agg[..., :Cc] / (agg[..., Cc:] + 1e-16)
    out = out.reshape(GPC, Hh, NPG, Cc).transpose(0, 2, 1, 3).reshape(NL, Hh, Cc)
    out = out.reshape(NL, Hh * Cc) if concat else out.mean(axis=1)
    return out + bias


def _build_adj(src, dst):
    """[GPC, NPG, NPG] multiplicity matrix: A[g, d, s] = #edges s->d + self loop."""
    g = dst // NPG
    flat = g * (NPG * NPG) + (dst - g * NPG) * NPG + (src - g * NPG)
    A = jnp.zeros((GPC * NPG * NPG,), jnp.float32).at[flat].add(1.0)
    return A.reshape(GPC, NPG, NPG) + jnp.eye(NPG, dtype=jnp.float32)[None]


def _forward(x, A_fwd, A_bwd, actions_idx, tabu_label,
             f1_W, f1_as, f1_ad, f1_b, f2_W, f2_as, f2_ad, f2_b,
             f3_W, f3_as, f3_ad, f3_b, b1_W, b1_as, b1_ad, b1_b,
             b2_W, b2_as, b2_ad, b2_b, b3_W, b3_as, b3_ad, b3_b,
             p_W1, p_b1, p_W2, p_b2, p_W3, p_b3, p_W4, p_b4):
    hf = x[:, jnp.array([0, 1, 3])]
    hf = jax.nn.elu(_gat_dense(hf, A_fwd, f1_W, f1_as, f1_ad, f1_b, True))
    hf = jax.nn.elu(_gat_dense(hf, A_fwd, f2_W, f2_as, f2_ad, f2_b, True))
    hf3 = _gat_dense(hf, A_fwd, f3_W, f3_as, f3_ad, f3_b, False)
    hb = x[:, jnp.array([0, 2, 4])]
    hb = jax.nn.elu(_gat_dense(hb, A_bwd, b1_W, b1_as, b1_ad, b1_b, True))
    hb = jax.nn.elu(_gat_dense(hb, A_bwd, b2_W, b2_as, b2_ad, b2_b, True))
    hb3 = _gat_dense(hb, A_bwd, b3_W, b3_as, b3_ad, b3_b, False)
    h_node = jnp.concatenate([hf3, hb3], axis=-1)          # [NL, 2*OUT]
    g_pool = h_node.reshape(GPC, NPG, 2 * OUT).mean(axis=1)
    g_exp = jnp.repeat(g_pool, NPG, axis=0)                # graphs contiguous
    node_h = jnp.concatenate([h_node, g_exp], axis=-1)     # [NL, 4*OUT]
    action_h = jnp.concatenate([node_h[actions_idx[:, 0]],
                                node_h[actions_idx[:, 1]],
                                tabu_label], axis=-1)
    h = jnp.tanh(action_h @ p_W1 + p_b1)
    h = jnp.tanh(h @ p_W2 + p_b2)
    h = jnp.tanh(h @ p_W3 + p_b3)
    score = (h @ p_W4 + p_b4).reshape(GPC, APG)
    log_prob = jax.nn.log_softmax(score, axis=-1)
    entropy = -(jnp.exp(log_prob) * log_prob).sum(-1, keepdims=True)
    return log_prob, entropy


_WNAMES = ['f1_W', 'f1_as', 'f1_ad', 'f1_b', 'f2_W', 'f2_as', 'f2_ad', 'f2_b',
           'f3_W', 'f3_as', 'f3_ad', 'f3_b', 'b1_W', 'b1_as', 'b1_ad', 'b1_b',
           'b2_W', 'b2_as', 'b2_ad', 'b2_b', 'b3_W', 'b3_as', 'b3_ad', 'b3_b',
           'p_W1', 'p_b1', 'p_W2', 'p_b2', 'p_W3', 'p_b3', 'p_W4', 'p_b4']

_pF = None
_pA = None


def _get_fns():
    global _pF, _pA
    if _pF is None:
        devs = jax.devices()[:M]
        _pA = jax.pmap(lambda s, d: (_build_adj(s, d), _build_adj(d, s)),
                       devices=devs)
        _pF = jax.pmap(
            _forward,
            in_axes=(0, 0, 0, 0, 0) + (None,) * 32,
            devices=devs,
        )
    return _pA, _pF


def kernel(**inputs):
    x = np.asarray(inputs['x']).reshape(M, NL, 5)
    ei = np.asarray(inputs['edge_index'])
    base = (np.arange(M, dtype=np.int32) * NL)[:, None]
    src = (ei[0].reshape(M, EL) - base).astype(np.int32)
    dst = (ei[1].reshape(M, EL) - base).astype(np.int32)
    act = (np.asarray(inputs['actions_idx']).reshape(M, AL, 2)
           - base[:, :, None]).astype(np.int32)
    tabu = np.asarray(inputs['tabu_label']).reshape(M, AL, 1)
    weights = [np.asarray(inputs[k]) for k in _WNAMES]

    pA, pF = _get_fns()
    A_fwd, A_bwd = pA(src, dst)     # device-resident, never pulled to host
    log_prob, entropy = pF(x, A_fwd, A_bwd, act, tabu, *weights)
    log_prob = np.asarray(log_prob).reshape(B, APG)
    entropy = np.asarray(entropy).reshape(B, 1)
    return log_prob, entropy


# revision 7
# speedup vs baseline: 1.1125x; 1.0545x over previous
import numpy as np
import jax
import jax.numpy as jnp

# Problem shapes (hardcoded per spec)
B = 64       # graphs
NPG = 800    # nodes per graph
EPG = 6400   # edges per graph
APG = 64     # actions per graph
H = 4        # heads
C = 64       # channels per head
OUT = 128    # out channels
M = 8        # cores
GPC = B // M         # graphs per core = 8
NL = GPC * NPG       # nodes per core = 6400
EL = GPC * EPG       # edges per core = 51200
AL = GPC * APG       # actions per core = 512


def _gat_dense(h, logA, W, a_src, a_dst, bias, concat):
    """Dense per-graph GAT, exact (multiplicity-weighted, shift-invariant softmax).

    h:    [NL, fin] node features (graphs contiguous blocks of NPG)
    logA: [GPC, NPG, NPG] log multiplicity, -inf where no edge (incl. self loops)
    """
    Hh, Cc = a_src.shape
    GB = GPC * Hh
    xp = (h @ W).reshape(NL, Hh, Cc)                       # [NL,H,C]
    al_s = (xp * a_src).sum(-1).reshape(GPC, NPG, Hh)      # [G,N,H]
    al_d = (xp * a_dst).sum(-1).reshape(GPC, NPG, Hh)
    alsT = al_s.transpose(0, 2, 1).reshape(GPC, Hh, 1, NPG)   # [G,H,1,S]
    aldT = al_d.transpose(0, 2, 1).reshape(GPC, Hh, NPG, 1)   # [G,H,D,1]
    # per-dst upper bound on the row logits: mhat >= lr(al_s[s]+al_d[d]) for all s
    gms = al_s.max(axis=1).reshape(GPC, Hh, 1, 1)
    mhat = jax.nn.leaky_relu(gms + aldT, 0.2)              # [G,H,D,1]
    z = jax.nn.leaky_relu(aldT + alsT, 0.2)                # [G,H,D,S]
    ex = jnp.exp(z - mhat + logA[:, None])                 # <= count, 0 off-edges
    ex = ex.reshape(GB, NPG, NPG)
    # aggregate: out[b,d,c] = sum_s ex[b,d,s] xp[b,s,c]; ones column -> denominator
    xph = xp.reshape(GPC, NPG, Hh, Cc).transpose(0, 2, 1, 3).reshape(GB, NPG, Cc)
    xph1 = jnp.concatenate([xph, jnp.ones((GB, NPG, 1), xph.dtype)], axis=-1)
    agg = jnp.matmul(ex.astype(jnp.bfloat16), xph1.astype(jnp.bfloat16),
                     preferred_element_type=jnp.float32)   # [GB,D,C+1]
    out = agg[..., :Cc] / (agg[..., Cc:] + 1e-16)
    out = out.reshape(GPC, Hh, NPG, Cc).transpose(0, 2, 1, 3).reshape(NL, Hh, Cc)
    out = out.reshape(NL, Hh * Cc) if concat else out.mean(axis=1)
    return out + bias


def _build_adj(src, dst):
    """[GPC, NPG, NPG] log-multiplicity: log(#edges s->d + self loop), -inf if none.

    Built scatter-free: one-hot matmul counts (exact small integers)."""
    gsz = NPG * NPG
    srcg = src.reshape(GPC, EPG) - (jnp.arange(GPC, dtype=src.dtype) * NPG)[:, None]
    dstg = dst.reshape(GPC, EPG) - (jnp.arange(GPC, dtype=src.dtype) * NPG)[:, None]
    ar = jnp.arange(NPG, dtype=src.dtype)
    soh = (srcg[:, :, None] == ar).astype(jnp.bfloat16)    # [G,E,S]
    doh = (dstg[:, :, None] == ar).astype(jnp.bfloat16)    # [G,E,D]
    A = jnp.einsum('ged,ges->gds', doh, soh,
                   preferred_element_type=jnp.float32)
    A = A + jnp.eye(NPG, dtype=jnp.float32)[None]
    return jnp.log(A)


def _forward(x, A_fwd, A_bwd, actions_idx, tabu_label,
             f1_W, f1_as, f1_ad, f1_b, f2_W, f2_as, f2_ad, f2_b,
             f3_W, f3_as, f3_ad, f3_b, b1_W, b1_as, b1_ad, b1_b,
             b2_W, b2_as, b2_ad, b2_b, b3_W, b3_as, b3_ad, b3_b,
             p_W1, p_b1, p_W2, p_b2, p_W3, p_b3, p_W4, p_b4):
    hf = x[:, jnp.array([0, 1, 3])]
    hf = jax.nn.elu(_gat_dense(hf, A_fwd, f1_W, f1_as, f1_ad, f1_b, True))
    hf = jax.nn.elu(_gat_dense(hf, A_fwd, f2_W, f2_as, f2_ad, f2_b, True))
    hf3 = _gat_dense(hf, A_fwd, f3_W, f3_as, f3_ad, f3_b, False)
    hb = x[:, jnp.array([0, 2, 4])]
    hb = jax.nn.elu(_gat_dense(hb, A_bwd, b1_W, b1_as, b1_ad, b1_b, True))
    hb = jax.nn.elu(_gat_dense(hb, A_bwd, b2_W, b2_as, b2_ad, b2_b, True))
    hb3 = _gat_dense(hb, A_bwd, b3_W, b3_as, b3_ad, b3_b, False)
    h_node = jnp.concatenate([hf3, hb3], axis=-1)          # [NL, 2*OUT]
    g_pool = h_node.reshape(GPC, NPG, 2 * OUT).mean(axis=1)
    g_exp = jnp.repeat(g_pool, NPG, axis=0)                # graphs contiguous
    node_h = jnp.concatenate([h_node, g_exp], axis=-1)     # [NL, 4*OUT]
    action_h = jnp.concatenate([node_h[actions_idx[:, 0]],
                                node_h[actions_idx[:, 1]],
                                tabu_label], axis=-1)
    h = jnp.tanh(action_h @ p_W1 + p_b1)
    h = jnp.tanh(h @ p_W2 + p_b2)
    h = jnp.tanh(h @ p_W3 + p_b3)
    score = (h @ p_W4 + p_b4).reshape(GPC, APG)
    log_prob = jax.nn.log_softmax(score, axis=-1)
    entropy = -(jnp.exp(log_prob) * log_prob).sum(-1, keepdims=True)
    return log_prob, entropy


_WNAMES = ['f1_W', 'f1_as', 'f1_ad', 'f1_b', 'f2_W', 'f2_as', 'f2_ad', 'f2_b',
           'f3_W', 'f3_as', 'f3_ad', 'f3_b', 'b1_W', 'b1_as', 'b1_ad', 'b1_b',
           'b2_W', 'b2_as', 'b2_ad', 'b2_b', 'b3_W', 'b3_as', 'b3_ad', 'b3_b',
           'p_W1', 'p_b1', 'p_W2', 'p_b2', 'p_W3', 'p_b3', 'p_W4', 'p_b4']

_pF = None
_pA = None


def _get_fns():
    global _pF, _pA
    if _pF is None:
        devs = jax.devices()[:M]
        _pA = jax.pmap(lambda s, d: (_build_adj(s, d), _build_adj(d, s)),
                       devices=devs)
        _pF = jax.pmap(
            _forward,
            in_axes=(0, 0, 0, 0, 0) + (None,) * 32,
            devices=devs,
        )
    return _pA, _pF


def kernel(**inputs):
    x = np.asarray(inputs['x']).reshape(M, NL, 5)
    ei = np.asarray(inputs['edge_index'])
    base = (np.arange(M, dtype=np.int32) * NL)[:, None]
    src = (ei[0].reshape(M, EL) - base).astype(np.int32)
    dst = (ei[1].reshape(M, EL) - base).astype(np.int32)
    act = (np.asarray(inputs['actions_idx']).reshape(M, AL, 2)
           - base[:, :, None]).astype(np.int32)
    tabu = np.asarray(inputs['tabu_label']).reshape(M, AL, 1)
    weights = [np.asarray(inputs[k]) for k in _WNAMES]

    pA, pF = _get_fns()
    A_fwd, A_bwd = pA(src, dst)     # device-resident, never pulled to host
    log_prob, entropy = pF(x, A_fwd, A_bwd, act, tabu, *weights)
    log_prob = np.asarray(log_prob).reshape(B, APG)
    entropy = np.asarray(entropy).reshape(B, 1)
    return log_prob, entropy


# revision 8
# speedup vs baseline: 2.3306x; 2.0950x over previous
import numpy as np
import jax
import jax.numpy as jnp

# Problem shapes (hardcoded per spec)
B = 64       # graphs
NPG = 800    # nodes per graph
EPG = 6400   # edges per graph
APG = 64     # actions per graph
H = 4        # heads
C = 64       # channels per head
OUT = 128    # out channels
M = 8        # cores
GPC = B // M         # graphs per core = 8
NL = GPC * NPG       # nodes per core = 6400
EL = GPC * EPG       # edges per core = 51200
AL = GPC * APG       # actions per core = 512


def _gat_dense(h, logA, W, a_src, a_dst, bias, concat):
    """Dense per-graph GAT, exact (multiplicity-weighted, shift-invariant softmax).

    h:    [NL, fin] node features (graphs contiguous blocks of NPG)
    logA: [GPC, NPG, NPG] log multiplicity, -inf where no edge (incl. self loops)
    """
    Hh, Cc = a_src.shape
    GB = GPC * Hh
    xp = (h @ W).reshape(NL, Hh, Cc)                       # [NL,H,C]
    al_s = (xp * a_src).sum(-1).reshape(GPC, NPG, Hh)      # [G,N,H]
    al_d = (xp * a_dst).sum(-1).reshape(GPC, NPG, Hh)
    alsT = al_s.transpose(0, 2, 1).reshape(GPC, Hh, 1, NPG)   # [G,H,1,S]
    aldT = al_d.transpose(0, 2, 1).reshape(GPC, Hh, NPG, 1)   # [G,H,D,1]
    # per-dst upper bound on the row logits: mhat >= lr(al_s[s]+al_d[d]) for all s
    gms = al_s.max(axis=1).reshape(GPC, Hh, 1, 1)
    mhat = jax.nn.leaky_relu(gms + aldT, 0.2)              # [G,H,D,1]
    z = jax.nn.leaky_relu(aldT + alsT, 0.2)                # [G,H,D,S]
    ex = jnp.exp(z - mhat + logA[:, None])                 # <= count, 0 off-edges
    ex = ex.reshape(GB, NPG, NPG)
    # aggregate: out[b,d,c] = sum_s ex[b,d,s] xp[b,s,c]; ones column -> denominator
    xph = xp.reshape(GPC, NPG, Hh, Cc).transpose(0, 2, 1, 3).reshape(GB, NPG, Cc)
    xph1 = jnp.concatenate([xph, jnp.ones((GB, NPG, 1), xph.dtype)], axis=-1)
    agg = jnp.matmul(ex.astype(jnp.bfloat16), xph1.astype(jnp.bfloat16),
                     preferred_element_type=jnp.float32)   # [GB,D,C+1]
    out = agg[..., :Cc] / (agg[..., Cc:] + 1e-16)
    out = out.reshape(GPC, Hh, NPG, Cc).transpose(0, 2, 1, 3).reshape(NL, Hh, Cc)
    out = out.reshape(NL, Hh * Cc) if concat else out.mean(axis=1)
    return out + bias


def _build_logA(src, dst):
    """[GPC, NPG, NPG] log multiplicity of forward adjacency (A[g,d,s], + self loops).

    Scatter-free: one-hot matmul counting (exact small integers)."""
    srcg = src.reshape(GPC, EPG) - (jnp.arange(GPC, dtype=src.dtype) * NPG)[:, None]
    dstg = dst.reshape(GPC, EPG) - (jnp.arange(GPC, dtype=src.dtype) * NPG)[:, None]
    ar = jnp.arange(NPG, dtype=src.dtype)
    soh = (srcg[:, :, None] == ar).astype(jnp.bfloat16)    # [G,E,S]
    doh = (dstg[:, :, None] == ar).astype(jnp.bfloat16)    # [G,E,D]
    A = jnp.einsum('ged,ges->gds', doh, soh,
                   preferred_element_type=jnp.float32)
    A = A + jnp.eye(NPG, dtype=jnp.float32)[None]
    return jnp.log(A)


def _forward(x, src, dst, actions_idx, tabu_label,
             f1_W, f1_as, f1_ad, f1_b, f2_W, f2_as, f2_ad, f2_b,
             f3_W, f3_as, f3_ad, f3_b, b1_W, b1_as, b1_ad, b1_b,
             b2_W, b2_as, b2_ad, b2_b, b3_W, b3_as, b3_ad, b3_b,
             p_W1, p_b1, p_W2, p_b2, p_W3, p_b3, p_W4, p_b4):
    logA_f = _build_logA(src, dst)
    logA_b = logA_f.transpose(0, 2, 1)     # reversed edges = transposed adjacency
    hf = x[:, jnp.array([0, 1, 3])]
    hf = jax.nn.elu(_gat_dense(hf, logA_f, f1_W, f1_as, f1_ad, f1_b, True))
    hf = jax.nn.elu(_gat_dense(hf, logA_f, f2_W, f2_as, f2_ad, f2_b, True))
    hf3 = _gat_dense(hf, logA_f, f3_W, f3_as, f3_ad, f3_b, False)
    hb = x[:, jnp.array([0, 2, 4])]
    hb = jax.nn.elu(_gat_dense(hb, logA_b, b1_W, b1_as, b1_ad, b1_b, True))
    hb = jax.nn.elu(_gat_dense(hb, logA_b, b2_W, b2_as, b2_ad, b2_b, True))
    hb3 = _gat_dense(hb, logA_b, b3_W, b3_as, b3_ad, b3_b, False)
    h_node = jnp.concatenate([hf3, hb3], axis=-1)          # [NL, 2*OUT]
    g_pool = h_node.reshape(GPC, NPG, 2 * OUT).mean(axis=1)
    g_exp = jnp.repeat(g_pool, NPG, axis=0)                # graphs contiguous
    node_h = jnp.concatenate([h_node, g_exp], axis=-1)     # [NL, 4*OUT]
    action_h = jnp.concatenate([node_h[actions_idx[:, 0]],
                                node_h[actions_idx[:, 1]],
                                tabu_label], axis=-1)
    h = jnp.tanh(action_h @ p_W1 + p_b1)
    h = jnp.tanh(h @ p_W2 + p_b2)
    h = jnp.tanh(h @ p_W3 + p_b3)
    score = (h @ p_W4 + p_b4).reshape(GPC, APG)
    log_prob = jax.nn.log_softmax(score, axis=-1)
    entropy = -(jnp.exp(log_prob) * log_prob).sum(-1, keepdims=True)
    return log_prob, entropy


_WNAMES = ['f1_W', 'f1_as', 'f1_ad', 'f1_b', 'f2_W', 'f2_as', 'f2_ad', 'f2_b',
           'f3_W', 'f3_as', 'f3_ad', 'f3_b', 'b1_W', 'b1_as', 'b1_ad', 'b1_b',
           'b2_W', 'b2_as', 'b2_ad', 'b2_b', 'b3_W', 'b3_as', 'b3_ad', 'b3_b',
           'p_W1', 'p_b1', 'p_W2', 'p_b2', 'p_W3', 'p_b3', 'p_W4', 'p_b4']

_pF = None
_wcache = {}


def _get_fn():
    global _pF
    if _pF is None:
        _pF = jax.pmap(_forward, devices=jax.devices()[:M])  # all in_axes=0
    return _pF


def _dev_weights(inputs):
    """Replicate weights to all devices once; cache across calls."""
    key = tuple(id(inputs[k]) for k in _WNAMES)
    if key not in _wcache:
        devs = jax.devices()[:M]
        _wcache.clear()
        _wcache[key] = [jax.device_put_replicated(np.asarray(inputs[k]), devs)
                        for k in _WNAMES]
    return _wcache[key]


def kernel(**inputs):
    x = np.asarray(inputs['x']).reshape(M, NL, 5)
    ei = np.asarray(inputs['edge_index'])
    base = (np.arange(M, dtype=np.int32) * NL)[:, None]
    src = (ei[0].reshape(M, EL) - base).astype(np.int32)
    dst = (ei[1].reshape(M, EL) - base).astype(np.int32)
    act = (np.asarray(inputs['actions_idx']).reshape(M, AL, 2)
           - base[:, :, None]).astype(np.int32)
    tabu = np.asarray(inputs['tabu_label']).reshape(M, AL, 1)
    weights = _dev_weights(inputs)

    log_prob, entropy = _get_fn()(x, src, dst, act, tabu, *weights)
    log_prob = np.asarray(log_prob).reshape(B, APG)
    entropy = np.asarray(entropy).reshape(B, 1)
    return log_prob, entropy


# revision 11
# speedup vs baseline: 3.0415x; 1.3050x over previous
import numpy as np
import jax
import jax.numpy as jnp

# Problem shapes (hardcoded per spec)
B = 64       # graphs
NPG = 800    # nodes per graph
EPG = 6400   # edges per graph
APG = 64     # actions per graph
H = 4        # heads
C = 64       # channels per head
OUT = 128    # out channels
M = 8        # cores
GPC = B // M         # graphs per core = 8
NL = GPC * NPG       # nodes per core = 6400
EL = GPC * EPG       # edges per core = 51200
AL = GPC * APG       # actions per core = 512


def _gat_dense(h, logA, W, a_src, a_dst, bias, concat):
    """Dense per-graph GAT, exact (multiplicity-weighted, shift-invariant softmax).

    h:    [NL, fin] node features (graphs contiguous blocks of NPG)
    logA: [GPC, NPG, NPG] log multiplicity, -inf where no edge (incl. self loops)
    """
    Hh, Cc = a_src.shape
    GB = GPC * Hh
    xp = (h @ W).reshape(NL, Hh, Cc)                       # [NL,H,C]
    al_s = (xp * a_src).sum(-1).reshape(GPC, NPG, Hh)      # [G,N,H]
    al_d = (xp * a_dst).sum(-1).reshape(GPC, NPG, Hh)
    alsT = al_s.transpose(0, 2, 1).reshape(GPC, Hh, 1, NPG)   # [G,H,1,S]
    aldT = al_d.transpose(0, 2, 1).reshape(GPC, Hh, NPG, 1)   # [G,H,D,1]
    # per-dst upper bound on the row logits: mhat >= lr(al_s[s]+al_d[d]) for all s
    gms = al_s.max(axis=1).reshape(GPC, Hh, 1, 1)
    mhat = jax.nn.leaky_relu(gms + aldT, 0.2)              # [G,H,D,1]
    z = jax.nn.leaky_relu(aldT + alsT, 0.2)                # [G,H,D,S]
    ex = jnp.exp(z - mhat + logA[:, None])                 # <= count, 0 off-edges
    ex = ex.reshape(GB, NPG, NPG)
    # aggregate: out[b,d,c] = sum_s ex[b,d,s] xp[b,s,c]; ones column -> denominator
    xph = xp.reshape(GPC, NPG, Hh, Cc).transpose(0, 2, 1, 3).reshape(GB, NPG, Cc)
    xph1 = jnp.concatenate([xph, jnp.ones((GB, NPG, 1), xph.dtype)], axis=-1)
    agg = jnp.matmul(ex.astype(jnp.bfloat16), xph1.astype(jnp.bfloat16),
                     preferred_element_type=jnp.float32)   # [GB,D,C+1]
    out = agg[..., :Cc] / (agg[..., Cc:] + 1e-16)
    out = out.reshape(GPC, Hh, NPG, Cc).transpose(0, 2, 1, 3).reshape(NL, Hh, Cc)
    out = out.reshape(NL, Hh * Cc) if concat else out.mean(axis=1)
    return out + bias


def _build_logA(src, dst):
    """[GPC, NPG, NPG] log multiplicity of forward adjacency (A[g,d,s], + self loops).

    Scatter-free: one-hot matmul counting (exact small integers)."""
    srcg = src.reshape(GPC, EPG)           # int16, graph-local [0, NPG)
    dstg = dst.reshape(GPC, EPG)
    ar = jnp.arange(NPG, dtype=src.dtype)
    soh = (srcg[:, :, None] == ar).astype(jnp.bfloat16)    # [G,E,S]
    doh = (dstg[:, :, None] == ar).astype(jnp.bfloat16)    # [G,E,D]
    A = jnp.einsum('ged,ges->gds', doh, soh,
                   preferred_element_type=jnp.float32)
    A = A + jnp.eye(NPG, dtype=jnp.float32)[None]
    return jnp.log(A)


def _forward(x, src, dst, actions_idx, tabu_label,
             f1_W, f1_as, f1_ad, f1_b, f2_W, f2_as, f2_ad, f2_b,
             f3_W, f3_as, f3_ad, f3_b, b1_W, b1_as, b1_ad, b1_b,
             b2_W, b2_as, b2_ad, b2_b, b3_W, b3_as, b3_ad, b3_b,
             p_W1, p_b1, p_W2, p_b2, p_W3, p_b3, p_W4, p_b4):
    logA_f = _build_logA(src, dst)
    logA_b = logA_f.transpose(0, 2, 1)     # reversed edges = transposed adjacency
    hf = x[:, jnp.array([0, 1, 3])]
    hf = jax.nn.elu(_gat_dense(hf, logA_f, f1_W, f1_as, f1_ad, f1_b, True))
    hf = jax.nn.elu(_gat_dense(hf, logA_f, f2_W, f2_as, f2_ad, f2_b, True))
    hf3 = _gat_dense(hf, logA_f, f3_W, f3_as, f3_ad, f3_b, False)
    hb = x[:, jnp.array([0, 2, 4])]
    hb = jax.nn.elu(_gat_dense(hb, logA_b, b1_W, b1_as, b1_ad, b1_b, True))
    hb = jax.nn.elu(_gat_dense(hb, logA_b, b2_W, b2_as, b2_ad, b2_b, True))
    hb3 = _gat_dense(hb, logA_b, b3_W, b3_as, b3_ad, b3_b, False)
    h_node = jnp.concatenate([hf3, hb3], axis=-1)          # [NL, 2*OUT]
    g_pool = h_node.reshape(GPC, NPG, 2 * OUT).mean(axis=1)
    g_exp = jnp.repeat(g_pool, NPG, axis=0)                # graphs contiguous
    node_h = jnp.concatenate([h_node, g_exp], axis=-1)     # [NL, 4*OUT]
    ai = actions_idx.astype(jnp.int32)
    action_h = jnp.concatenate([node_h[ai[:, 0]],
                                node_h[ai[:, 1]],
                                tabu_label], axis=-1)
    h = jnp.tanh(action_h @ p_W1 + p_b1)
    h = jnp.tanh(h @ p_W2 + p_b2)
    h = jnp.tanh(h @ p_W3 + p_b3)
    score = (h @ p_W4 + p_b4).reshape(GPC, APG)
    log_prob = jax.nn.log_softmax(score, axis=-1)
    entropy = -(jnp.exp(log_prob) * log_prob).sum(-1, keepdims=True)
    return log_prob, entropy


_WNAMES = ['f1_W', 'f1_as', 'f1_ad', 'f1_b', 'f2_W', 'f2_as', 'f2_ad', 'f2_b',
           'f3_W', 'f3_as', 'f3_ad', 'f3_b', 'b1_W', 'b1_as', 'b1_ad', 'b1_b',
           'b2_W', 'b2_as', 'b2_ad', 'b2_b', 'b3_W', 'b3_as', 'b3_ad', 'b3_b',
           'p_W1', 'p_b1', 'p_W2', 'p_b2', 'p_W3', 'p_b3', 'p_W4', 'p_b4']

_pF = None
_wcache = {}


def _get_fn():
    global _pF
    if _pF is None:
        _pF = jax.pmap(_forward, devices=jax.devices()[:M])  # all in_axes=0
    return _pF


def _dev_weights(inputs):
    """Replicate weights to all devices once; cache across calls."""
    key = tuple(id(inputs[k]) for k in _WNAMES)
    if key not in _wcache:
        devs = jax.devices()[:M]
        _wcache.clear()
        _wcache[key] = [jax.device_put_replicated(np.asarray(inputs[k]), devs)
                        for k in _WNAMES]
    return _wcache[key]


def kernel(**inputs):
    x = np.asarray(inputs['x']).reshape(M, NL, 5)
    ei = np.asarray(inputs['edge_index'])
    gbase = (np.arange(B, dtype=np.int32) * NPG).repeat(EPG)  # per-edge graph base
    src = (ei[0] - gbase).astype(np.int16).reshape(M, EL)     # graph-local [0, NPG)
    dst = (ei[1] - gbase).astype(np.int16).reshape(M, EL)
    base = (np.arange(M, dtype=np.int32) * NL)[:, None]
    act = (np.asarray(inputs['actions_idx']).reshape(M, AL, 2)
           - base[:, :, None]).astype(np.int16)               # core-local [0, NL)
    tabu = np.asarray(inputs['tabu_label']).reshape(M, AL, 1)
    weights = _dev_weights(inputs)

    log_prob, entropy = _get_fn()(x, src, dst, act, tabu, *weights)
    log_prob = np.asarray(log_prob).reshape(B, APG)
    entropy = np.asarray(entropy).reshape(B, 1)
    return log_prob, entropy
